# revision 1
# baseline (speedup 1.0000x reference)
"""Butterfly (nn_Butterfly) kernel for 8 Trainium2 NeuronCores.

Math: the 10-stage butterfly over n=1024 composes to a dense 1024x1024
matrix W (out = x @ W.T + bias).  We shard the batch (32768) across 8
cores; each core computes out^T = W @ x^T as a feature-major matmul in
bf16 with f32 PSUM accumulation.  The host pre-transposes x per shard
(so the device needs no transposes at all) and un-transposes the
feature-major output.
"""

import os
import numpy as np
import ml_dtypes

import concourse.bass as bass
import concourse.bacc as bacc
import concourse.mybir as mybir
import concourse.tile as tile
from concourse.bass_utils import run_bass_kernel_spmd

N_FEAT = 1024
M_STAGES = 10
N_CORES = 8

BF16 = ml_dtypes.bfloat16

LAST_EXEC_NS = None  # set when BASS_KERNEL_TRACE=1


def _apply_stages(x, twiddle, blocks):
    """Apply butterfly stages `blocks` (list of stage indices) to x [b, 1024].

    Mirrors reference.butterfly_mult_untied for nstack=1, increasing_stride.
    """
    n = N_FEAT
    for m in blocks:
        s = 1 << m
        t = twiddle[0, m].reshape(n // (2 * s), s, 2, 2)
        o = x.reshape(-1, n // (2 * s), 2, s)
        x = np.einsum("gsij,bgjs->bgis", t, o).reshape(-1, n)
    return x


def _dense_w(twiddle):
    """Composite dense W [1024, 1024] with out = x @ W.T."""
    eye = np.eye(N_FEAT, dtype=np.float64)
    m = _apply_stages(eye, twiddle.astype(np.float64), range(M_STAGES))
    return m.T  # M[e, p'] = W[p', e]


_CACHE = {}


def _phase_mats(twiddle):
    """Host prep for the two-phase decomposition.

    Returns (wa, wb, perm) where
      wa [1024, 128] bf16: rows 128c..128c+127 = WA_c^T (lhsT for phase A tile c)
      wb [1024, 128] bf16: rows 128c'.. = WB_c'^T (lhsT for phase B tile c')
    Phase A: block-diag over contiguous 32-blocks (stages 0-4).
    Phase B acts on q-space q = 32r + a (p = 32a + r), block-diag there
    (stages 5-9).
    """
    tw = twiddle.astype(np.float64)
    eye = np.eye(N_FEAT, dtype=np.float64)
    a_full = _apply_stages(eye, tw, range(5)).T
    b_full = _apply_stages(eye, tw, range(5, 10)).T
    # Partition layouts chosen so the pi exchange is 64 plain [16, N] copies:
    #   sa tile c, partition s~ = 16*cp + 4u + w holds feature 128c + sigma[s~]
    #   saq tile cp, partition s~' = 16*c + 4u + w holds q-local tau[s~']
    st = np.arange(128)
    sigma = 32 * ((st >> 2) & 3) + 4 * (st >> 4) + (st & 3)
    tau = 32 * (st & 3) + 4 * (st >> 4) + ((st >> 2) & 3)
    wa = np.zeros((N_FEAT, 128), dtype=np.float64)
    wb = np.zeros((N_FEAT, 128), dtype=np.float64)
    for c in range(8):
        blk_a = np.zeros((128, 128))
        blk_b = np.zeros((128, 128))
        for i in range(4):
            g = 4 * c + i
            blk_a[i * 32:(i + 1) * 32, i * 32:(i + 1) * 32] = a_full[
                g * 32:(g + 1) * 32, g * 32:(g + 1) * 32
            ]
            r = 4 * c + i
            rows = 32 * np.arange(32) + r
            blk_b[i * 32:(i + 1) * 32, i * 32:(i + 1) * 32] = b_full[np.ix_(rows, rows)]
        wa[c * 128:(c + 1) * 128] = blk_a[sigma, :].T  # lhsT[j, s~]
        wb[c * 128:(c + 1) * 128] = blk_b[:, tau].T    # lhsT[k~, m]
    # device SBUF layout [128, 8*128]: partition j, free c*128+m
    wa_sb = np.ascontiguousarray(
        wa.reshape(8, 128, 128).transpose(1, 0, 2).reshape(128, 1024)
    )
    wb_sb = np.ascontiguousarray(
        wb.reshape(8, 128, 128).transpose(1, 0, 2).reshape(128, 1024)
    )
    return wa_sb.astype(BF16), wb_sb.astype(BF16)


def _biasq(bias):
    """[128, 8] f32; biasq[s', c'] = bias[32*(s'%32) + 4c' + s'//32]."""
    out = np.zeros((128, 8), dtype=np.float32)
    sp = np.arange(128)
    for cp in range(8):
        out[:, cp] = bias[32 * (sp % 32) + 4 * cp + sp // 32]
    return out


def _unpermute_out(outq):
    """outq [1024, bpc] q-major -> out [bpc, 1024] p-major."""
    bpc = outq.shape[1]
    return np.ascontiguousarray(
        outq.reshape(32, 32, bpc).transpose(2, 1, 0).reshape(bpc, N_FEAT)
    )


def _pack_xt(shard_bf, chunk):
    """[bpc, 1024] bf16 -> chunk-major device layout [128, 8*bpc]:
    xt[p, j*8*chunk + c*chunk + n] = x[j*chunk + n, 128c + p]."""
    bpc = shard_bf.shape[0]
    nch = bpc // chunk
    a = shard_bf.T.reshape(8, 128, nch, chunk)          # [c, p, j, n]
    return np.ascontiguousarray(
        a.transpose(1, 2, 0, 3).reshape(128, 8 * bpc)
    )


def _unpack_out(raw, chunk):
    """device out [128, 8*bpc] chunk-major (q-major features) -> [bpc, 1024]."""
    bpc = raw.shape[1] // 8
    nch = bpc // chunk
    outq = np.ascontiguousarray(
        raw.reshape(128, nch, 8, chunk).transpose(2, 0, 1, 3).reshape(N_FEAT, bpc)
    )
    return _unpermute_out(outq)


def _build_program_v2(bpc, chunk=2048, diag_pi=False, sa_bufs=None, saq_bufs=None,
                      pi_split=(1, 1, 2)):
    """Two-phase block-diagonal butterfly for one core's shard.

    Host pre-arranges xt/wa/wb in device SBUF layout (chunk-major), so each
    chunk is one plain contiguous load and one plain store. The pi exchange
    is either 64 simple [16, chunk] partition-offset DMAs or 15 merged
    diagonal DMAs (flat APs stepping partition+free together, ordered by
    explicit add_dep_helper edges).
    """
    from concourse.bass import AP as _AP
    from concourse.bass import _add_dep_helper

    assert bpc % chunk == 0 and chunk % 512 == 0
    nch = bpc // chunk
    nnb = chunk // 512
    nc = bacc.Bacc("TRN2", debug=False)
    xt_d = nc.dram_tensor("xt", [128, 8 * bpc], mybir.dt.bfloat16, kind="ExternalInput").ap()
    wa_d = nc.dram_tensor("wa", [128, 8 * 128], mybir.dt.bfloat16, kind="ExternalInput").ap()
    wb_d = nc.dram_tensor("wb", [128, 8 * 128], mybir.dt.bfloat16, kind="ExternalInput").ap()
    bias_d = nc.dram_tensor("biasq", [128, 8], mybir.dt.float32, kind="ExternalInput").ap()
    out_d = nc.dram_tensor("outqT", [128, 8 * bpc], mybir.dt.float32, kind="ExternalOutput").ap()

    big = chunk > 1024  # shallow buffering + per-cp stores when tiles are large
    if sa_bufs is None:
        sa_bufs = 1 if big else 2
    if saq_bufs is None:
        saq_bufs = 2
    with tile.TileContext(nc) as tc:
        with (
            tc.tile_pool(name="wpool", bufs=1) as w_pool,
            tc.tile_pool(name="xin", bufs=2) as xin_pool,
            tc.tile_pool(name="sa", bufs=sa_bufs) as sa_pool,
            tc.tile_pool(name="saq", bufs=saq_bufs) as saq_pool,
            tc.tile_pool(name="otile", bufs=3 if big else 2) as out_pool,
            tc.tile_pool(name="psa", bufs=2, space="PSUM") as psa_pool,
            tc.tile_pool(name="psb", bufs=4, space="PSUM") as psb_pool,
        ):
            wa_sb = w_pool.tile([128, 8 * 128], mybir.dt.bfloat16, name="wa_sb")
            wb_sb = w_pool.tile([128, 8 * 128], mybir.dt.bfloat16, name="wb_sb")
            bias_sb = w_pool.tile([128, 8], mybir.dt.float32, name="bias_sb")
            nc.scalar.dma_start(wa_sb[:], wa_d[:])
            nc.scalar.dma_start(wb_sb[:], wb_d[:])
            nc.scalar.dma_start(bias_sb[:], bias_d[:])

            for j in range(nch):
                xin = xin_pool.tile([128, 8 * chunk], mybir.dt.bfloat16, name="xin")
                nc.sync.dma_start(
                    xin[:], xt_d[:, j * 8 * chunk:(j + 1) * 8 * chunk]
                )
                sa = sa_pool.tile([128, 8 * chunk], mybir.dt.bfloat16, name="sa")
                copies = []
                for c in range(8):
                    nn = 0
                    while nn < nnb:
                        w = 2 if nn + 1 < nnb else 1  # pair matmuls per copy
                        ps = psa_pool.tile([128, 512 * w], mybir.dt.float32, name="psa_t")
                        for k in range(w):
                            nc.tensor.matmul(
                                ps[:, k * 512:(k + 1) * 512],
                                wa_sb[:, c * 128:(c + 1) * 128],
                                xin[:, c * chunk + (nn + k) * 512 : c * chunk + (nn + k + 1) * 512],
                                start=True,
                                stop=True,
                            )
                        copies.append(nc.vector.tensor_copy(
                            sa[:, c * chunk + nn * 512 : c * chunk + (nn + w) * 512],
                            ps[:],
                        ))
                        nn += w
                saq = saq_pool.tile([128, 8 * chunk], mybir.dt.bfloat16, name="saq")
                # pi partition-exchange (scatter absorbed into wa/wb row order):
                #   saq tile cp partitions [16c, 16c+16) <- sa tile c partitions [16cp, 16cp+16)
                engs = [nc.sync, nc.scalar, nc.gpsimd]
                pi_insts = []
                if diag_pi:
                    # diagonal d: pairs (cp, c = cp+d mod 8). Per wrap-free run,
                    # one DMA whose first dim advances 16 partitions AND one
                    # chunk of free space per step (flat element APs).
                    F = 8 * chunk
                    sa_h = sa[:].tensor
                    saq_h = saq[:].tensor
                    k = 0
                    for d in range(8):
                        runs = [(0, 8)] if d == 0 else [(0, 8 - d), (8 - d, 8)]
                        for lo, hi in runs:
                            cnt = hi - lo
                            if cnt <= 0:
                                continue
                            coff = d if lo == 0 else d - 8
                            src = _AP(
                                sa_h,
                                (16 * lo) * F + (lo + coff) * chunk,
                                [[16 * F + chunk, cnt], [F, 16], [1, chunk]],
                            )
                            dst = _AP(
                                saq_h,
                                (16 * (lo + coff)) * F + lo * chunk,
                                [[16 * F + chunk, cnt], [F, 16], [1, chunk]],
                            )
                            inst = engs[k % 3].dma_start(dst, src)
                            # Tile's range tracker can't see through these flat
                            # APs; order explicitly.
                            for cpy in copies:
                                _add_dep_helper(inst.ins, cpy.ins, sync=True,
                                                reason="pi reads all sa copies")
                            pi_insts.append(inst)
                            k += 1
                else:
                    # weighted round-robin over (sync, scalar, gpsimd)
                    sched = []
                    for e, wgt in zip(engs, pi_split):
                        sched += [e] * wgt
                    for cp in range(8):
                        for c in range(8):
                            eng = sched[(cp * 8 + c) % len(sched)]
                            pi_insts.append(eng.dma_start(
                                saq[16 * c : 16 * c + 16, cp * chunk:(cp + 1) * chunk],
                                sa[16 * cp : 16 * cp + 16, c * chunk:(c + 1) * chunk],
                            ))
                if not big:
                    ot = out_pool.tile([128, 8 * chunk], mybir.dt.float32, name="ot")
                for cp in range(8):
                    if big:
                        ot = out_pool.tile([128, chunk], mybir.dt.float32, name="ot")
                    for nn in range(nnb):
                        ps = psb_pool.tile([128, 512], mybir.dt.float32, name="psb_t")
                        mm = nc.tensor.matmul(
                            ps[:],
                            wb_sb[:, cp * 128:(cp + 1) * 128],
                            saq[:, cp * chunk + nn * 512 : cp * chunk + (nn + 1) * 512],
                            start=True,
                            stop=True,
                        )
                        if diag_pi:
                            for p_inst in pi_insts:
                                _add_dep_helper(mm.ins, p_inst.ins, sync=True,
                                                reason="phase B reads pi output")
                        off = (cp * chunk if not big else 0) + nn * 512
                        nc.scalar.activation(
                            ot[:, off : off + 512],
                            ps[:], mybir.ActivationFunctionType.Identity,
                            bias=bias_sb[:, cp : cp + 1],
                        )
                    if big:
                        nc.sync.dma_start(
                            out_d[:, j * 8 * chunk + cp * chunk : j * 8 * chunk + (cp + 1) * chunk],
                            ot[:],
                        )
                if not big:
                    nc.sync.dma_start(
                        out_d[:, j * 8 * chunk:(j + 1) * 8 * chunk], ot[:]
                    )

    nc.compile()
    return nc


def _build_program(bpc):
    """Build + compile the bass program for one core's shard [1024, bpc]."""
    nc = bacc.Bacc("TRN2", debug=False)
    xt_d = nc.dram_tensor("xt", [N_FEAT, bpc], mybir.dt.bfloat16, kind="ExternalInput").ap()
    wt_d = nc.dram_tensor("wt", [N_FEAT, N_FEAT], mybir.dt.bfloat16, kind="ExternalInput").ap()
    bias_d = nc.dram_tensor("biasr", [128, 8], mybir.dt.float32, kind="ExternalInput").ap()
    out_d = nc.dram_tensor("outT", [N_FEAT, bpc], mybir.dt.float32, kind="ExternalOutput").ap()

    NB = bpc // 512  # n-chunks of 512

    with tile.TileContext(nc) as tc:
        with (
            tc.tile_pool(name="xt", bufs=1) as xt_pool,
            tc.tile_pool(name="w", bufs=1) as w_pool,
            tc.tile_pool(name="bias", bufs=1) as b_pool,
            tc.tile_pool(name="out", bufs=8) as out_pool,
            tc.tile_pool(name="ps", bufs=8, space="PSUM") as ps_pool,
        ):
            # resident inputs
            xt_all = xt_pool.tile([128, 8 * bpc], mybir.dt.bfloat16, name="xt_all")
            w_all = w_pool.tile([128, 8 * N_FEAT], mybir.dt.bfloat16, name="w_all")
            bias_t = b_pool.tile([128, 8], mybir.dt.float32, name="bias_t")
            for k in range(8):
                nc.sync.dma_start(xt_all[:, k * bpc:(k + 1) * bpc], xt_d[k * 128:(k + 1) * 128, :])
                nc.sync.dma_start(w_all[:, k * N_FEAT:(k + 1) * N_FEAT], wt_d[k * 128:(k + 1) * 128, :])
            nc.sync.dma_start(bias_t[:], bias_d[:])

            for mi in range(8):
                for ni in range(NB):
                    ps = ps_pool.tile([128, 512], mybir.dt.float32, name="ps")
                    for k in range(8):
                        nc.tensor.matmul(
                            ps[:],
                            w_all[:, k * N_FEAT + 128 * mi : k * N_FEAT + 128 * mi + 128],
                            xt_all[:, k * bpc + ni * 512 : k * bpc + (ni + 1) * 512],
                            start=(k == 0),
                            stop=(k == 7),
                        )
                    ot = out_pool.tile([128, 512], mybir.dt.float32, name="ot")
                    if (mi + ni) % 2 == 0:
                        nc.scalar.activation(
                            ot[:], ps[:], mybir.ActivationFunctionType.Identity,
                            bias=bias_t[:, mi : mi + 1],
                        )
                    else:
                        nc.vector.tensor_scalar_add(ot[:], ps[:], bias_t[:, mi : mi + 1])
                    nc.sync.dma_start(out_d[mi * 128:(mi + 1) * 128, ni * 512:(ni + 1) * 512], ot[:])

    nc.compile()
    return nc


def _pick_chunk(bpc):
    for chunk in (2048, 1024, 512):
        if bpc % chunk == 0:
            return chunk
    raise ValueError(f"batch per core {bpc} must be a multiple of 512")


def kernel(x, twiddle, bias):
    global LAST_EXEC_NS
    batch = x.shape[0]
    assert batch % N_CORES == 0
    bpc = batch // N_CORES
    chunk = _pick_chunk(bpc)

    # ---- host prep ----
    wa, wb = _phase_mats(np.asarray(twiddle, dtype=np.float32))
    bq = _biasq(np.asarray(bias, dtype=np.float32))
    x_bf = np.asarray(x).astype(BF16)
    shards = [
        _pack_xt(x_bf[k * bpc:(k + 1) * bpc, :], chunk)  # [128, 8*bpc]
        for k in range(N_CORES)
    ]

    key = ("v2", bpc)
    if key not in _CACHE:
        _CACHE[key] = _build_program_v2(bpc, chunk=chunk)
    nc = _CACHE[key]

    in_maps = [
        {"xt": shards[k], "wa": wa, "wb": wb, "biasq": bq} for k in range(N_CORES)
    ]
    try:
        res = run_bass_kernel_spmd(nc, in_maps, core_ids=list(range(N_CORES)))
    except ModuleNotFoundError:
        # BASS_TRACE set but the axon NTFF hook module isn't installed in
        # this container; retry with tracing force-disabled.
        os.environ["BASS_NEVER_TRACE"] = "1"
        res = run_bass_kernel_spmd(nc, in_maps, core_ids=list(range(N_CORES)))
    if res.exec_time_ns is not None:
        LAST_EXEC_NS = res.exec_time_ns

    out = np.empty((batch, N_FEAT), dtype=np.float32)
    for k in range(N_CORES):
        out[k * bpc:(k + 1) * bpc, :] = _unpack_out(res.results[k]["outqT"], chunk)
    return out


def sim_time_ns(bpc=4096):
    """Deterministic single-core span from the instruction cost model
    (TimelineSim). All 8 cores run this same program in parallel."""
    from concourse.timeline_sim import TimelineSim

    key = ("v2", bpc)
    if key not in _CACHE:
        _CACHE[key] = _build_program_v2(bpc, chunk=_pick_chunk(bpc))
    return TimelineSim(_CACHE[key], trace=False).simulate()


def _build_null_program(bpc):
    """Same I/O signature as the real program, near-zero device work."""
    nc = bacc.Bacc("TRN2", debug=False)
    xt_d = nc.dram_tensor("xt", [N_FEAT, bpc], mybir.dt.bfloat16, kind="ExternalInput").ap()
    wt_d = nc.dram_tensor("wt", [N_FEAT, N_FEAT], mybir.dt.bfloat16, kind="ExternalInput").ap()
    bias_d = nc.dram_tensor("biasr", [128, 8], mybir.dt.float32, kind="ExternalInput").ap()
    out_d = nc.dram_tensor("outT", [N_FEAT, bpc], mybir.dt.float32, kind="ExternalOutput").ap()
    with tile.TileContext(nc) as tc:
        with tc.tile_pool(name="b", bufs=1) as pool:
            bias_t = pool.tile([128, 8], mybir.dt.float32, name="bias_t")
            nc.sync.dma_start(bias_t[:], bias_d[:])
            nc.sync.dma_start(out_d[0:128, 0:8], bias_t[:])
    nc.compile()
    return nc


def _measure_exec_ns(nc, in_maps, iters=(4, 36)):
    """Per-execution device time via the slope method.

    Builds the same sharded PJRT executable as run_bass_kernel_spmd's axon
    path, keeps inputs device-resident, chains executions by donating the
    previous call's output as the next call's (fully overwritten) output
    buffer, and fits wall(M2)-wall(M1) / (M2-M1).
    """
    import time
    import jax
    from jax.sharding import Mesh, PartitionSpec
    from jax.experimental.shard_map import shard_map
    from concourse import mybir as _mybir
    from concourse.bass2jax import (
        _bass_exec_p,
        install_neuronx_cc_hook,
        partition_id_tensor,
    )

    install_neuronx_cc_hook()

    partition_name = nc.partition_id_tensor.name if nc.partition_id_tensor else None
    in_names, out_names, out_avals = [], [], []
    for alloc in nc.m.functions[0].allocations:
        if not isinstance(alloc, _mybir.MemoryLocationSet):
            continue
        name = alloc.memorylocations[0].name
        if alloc.kind == "ExternalInput":
            if name != partition_name:
                in_names.append(name)
        elif alloc.kind == "ExternalOutput":
            out_names.append(name)
            out_avals.append(
                jax.core.ShapedArray(tuple(alloc.tensor_shape), _mybir.dt.np(alloc.dtype))
            )
    n_params = len(in_names)
    n_outs = len(out_avals)
    all_names = in_names + out_names
    if partition_name is not None:
        all_names = all_names + [partition_name]

    def _body(*args):
        operands = list(args)
        if partition_name is not None:
            operands.append(partition_id_tensor())
        outs = _bass_exec_p.bind(
            *operands,
            out_avals=tuple(out_avals),
            in_names=tuple(all_names),
            out_names=tuple(out_names),
            lowering_input_output_aliases=(),
            sim_require_finite=True,
            sim_require_nnan=True,
            nc=nc,
        )
        return tuple(outs)

    devices = jax.devices()[:N_CORES]
    mesh = Mesh(np.asarray(devices), ("core",))
    donate = tuple(range(n_params, n_params + n_outs))
    sharded = jax.jit(
        shard_map(
            _body,
            mesh=mesh,
            in_specs=(PartitionSpec("core"),) * (n_params + n_outs),
            out_specs=(PartitionSpec("core"),) * n_outs,
            check_rep=False,
        ),
        donate_argnums=donate,
        keep_unused=True,
    )

    concat_in = [
        np.concatenate([np.asarray(in_maps[c][nm]) for c in range(N_CORES)], axis=0)
        for nm in in_names
    ]
    zero = [
        np.zeros((N_CORES * av.shape[0], *av.shape[1:]), av.dtype) for av in out_avals
    ]
    sharding = jax.sharding.NamedSharding(mesh, PartitionSpec("core"))
    dev_in = [jax.device_put(a, sharding) for a in concat_in]

    def run_chain(m):
        outs = tuple(jax.device_put(z, sharding) for z in zero)
        t0 = time.time()
        for _ in range(m):
            outs = sharded(*dev_in, *outs)
        for o in outs:
            o.block_until_ready()
        return time.time() - t0

    run_chain(2)  # warm up compile + device
    m1, m2 = iters
    t1 = min(run_chain(m1) for _ in range(3))
    t2 = min(run_chain(m2) for _ in range(3))
    per_exec_ns = (t2 - t1) / (m2 - m1) * 1e9
    return per_exec_ns, t1, t2


def _measure_samples(nc, in_maps, n=30):
    """Wall-time n single executions (device-resident inputs); returns list of seconds."""
    import time
    import jax
    from jax.sharding import PartitionSpec

    sharded, dev_in, zero, sharding, meta = _build_sharded(nc, in_maps)
    samples = []
    outs = tuple(jax.device_put(z, sharding) for z in zero)
    for _ in range(3):  # warmup
        outs = sharded(*dev_in, *outs)
    for o in outs:
        o.block_until_ready()
    for _ in range(n):
        outs = tuple(jax.device_put(z, sharding) for z in zero)
        for o in outs:
            o.block_until_ready()
        t0 = time.time()
        outs = sharded(*dev_in, *outs)
        for o in outs:
            o.block_until_ready()
        samples.append(time.time() - t0)
    return samples


def _build_sharded(nc, in_maps):
    import jax
    from jax.sharding import Mesh, PartitionSpec
    from jax.experimental.shard_map import shard_map
    from concourse import mybir as _mybir
    from concourse.bass2jax import (
        _bass_exec_p,
        install_neuronx_cc_hook,
        partition_id_tensor,
    )

    install_neuronx_cc_hook()
    partition_name = nc.partition_id_tensor.name if nc.partition_id_tensor else None
    in_names, out_names, out_avals = [], [], []
    for alloc in nc.m.functions[0].allocations:
        if not isinstance(alloc, _mybir.MemoryLocationSet):
            continue
        name = alloc.memorylocations[0].name
        if alloc.kind == "ExternalInput":
            if name != partition_name:
                in_names.append(name)
        elif alloc.kind == "ExternalOutput":
            out_names.append(name)
            out_avals.append(
                jax.core.ShapedArray(tuple(alloc.tensor_shape), _mybir.dt.np(alloc.dtype))
            )
    n_params = len(in_names)
    n_outs = len(out_avals)
    all_names = in_names + out_names
    if partition_name is not None:
        all_names = all_names + [partition_name]

    def _body(*args):
        operands = list(args)
        if partition_name is not None:
            operands.append(partition_id_tensor())
        outs = _bass_exec_p.bind(
            *operands,
            out_avals=tuple(out_avals),
            in_names=tuple(all_names),
            out_names=tuple(out_names),
            lowering_input_output_aliases=(),
            sim_require_finite=True,
            sim_require_nnan=True,
            nc=nc,
        )
        return tuple(outs)

    devices = jax.devices()[:N_CORES]
    mesh = Mesh(np.asarray(devices), ("core",))
    donate = tuple(range(n_params, n_params + n_outs))
    sharded = jax.jit(
        shard_map(
            _body,
            mesh=mesh,
            in_specs=(PartitionSpec("core"),) * (n_params + n_outs),
            out_specs=(PartitionSpec("core"),) * n_outs,
            check_rep=False,
        ),
        donate_argnums=donate,
        keep_unused=True,
    )
    concat_in = [
        np.concatenate([np.asarray(in_maps[c][nm]) for c in range(N_CORES)], axis=0)
        for nm in in_names
    ]
    zero = [
        np.zeros((N_CORES * av.shape[0], *av.shape[1:]), av.dtype) for av in out_avals
    ]
    sharding = jax.sharding.NamedSharding(mesh, PartitionSpec("core"))
    dev_in = [jax.device_put(a, sharding) for a in concat_in]
    return sharded, dev_in, zero, sharding, (n_params, n_outs)


def bench(x, twiddle, bias, iters=(4, 36)):
    """Measure the kernel's device span: slope(real) - slope(null).

    NOTE: the axon RPC path is too noisy for this to be reliable
    (per-call jitter of 1-40 ms); prefer sim_time_ns() for optimization.
    """
    batch = x.shape[0]
    bpc = batch // N_CORES
    chunk = _pick_chunk(bpc)
    wa, wb = _phase_mats(np.asarray(twiddle, dtype=np.float32))
    bq = _biasq(np.asarray(bias, dtype=np.float32))
    x_bf = np.asarray(x).astype(BF16)
    shards = [
        _pack_xt(x_bf[k * bpc:(k + 1) * bpc, :], chunk) for k in range(N_CORES)
    ]
    in_maps = [
        {"xt": shards[k], "wa": wa, "wb": wb, "biasq": bq} for k in range(N_CORES)
    ]

    key = ("v2", bpc)
    if key not in _CACHE:
        _CACHE[key] = _build_program_v2(bpc, chunk=chunk)

    real_ns, rt1, rt2 = _measure_exec_ns(_CACHE[key], in_maps, iters)
    return real_ns, rt1, rt2



# revision 3
# speedup vs baseline: 1.7541x; 1.7541x over previous
"""Butterfly (nn_Butterfly) kernel for 8 Trainium2 NeuronCores.

Math: the 10 butterfly stages factor as out = B . (A . x) + bias where
A (stages 0-4) is block-diagonal over contiguous 32-blocks of p and
B (stages 5-9) is block-diagonal over contiguous 32-blocks of q, with
q = 32*(p % 32) + p//32.

Device pipeline per 128-sample block (three TensorE passes, no
SBUF<->SBUF DMA):
  pass A (flipped matmuls):  psa[samp, pfeat]  = x_blk^T . A^T   (per tile c)
  copyA (permuting):         sa2[samp, qfeat]  = psa permuted    (bf16)
  pass T (transposes):       pst[qfeat, samp]  = sa2^T           (per tile cp)
  copyT:                     saq[qfeat, samp]  (chunk-grouped)
  pass B (matmuls):          psb[qout, samp]   = B_cp . saq_cp
  copyB (+bias):             ot -> DMA out (bf16)

Batch (32768) is sharded across 8 cores; weights are replicated.
The host packs x into the transposed block-major device layout and
unpacks/un-permutes the q-major bf16 output back to f32 [batch, 1024].
"""

import os
import numpy as np
import ml_dtypes

import concourse.bass as bass
import concourse.bacc as bacc
import concourse.mybir as mybir
import concourse.tile as tile
from concourse.bass_utils import run_bass_kernel_spmd

N_FEAT = 1024
M_STAGES = 10
N_CORES = 8
NB = 4          # blocks per chunk
CH = NB * 128   # samples per chunk

BF16 = ml_dtypes.bfloat16

LAST_EXEC_NS = None  # set when BASS_KERNEL_TRACE=1


# ---------------------------------------------------------------------------
# host-side weight/layout prep
# ---------------------------------------------------------------------------

def _apply_stages(x, twiddle, blocks):
    """Apply butterfly stages `blocks` to x [b, 1024] (mirrors reference)."""
    n = N_FEAT
    for m in blocks:
        s = 1 << m
        t = twiddle[0, m].reshape(n // (2 * s), s, 2, 2)
        o = x.reshape(-1, n // (2 * s), 2, s)
        x = np.einsum("gsij,bgjs->bgis", t, o).reshape(-1, n)
    return x


def _q_perm():
    """p -> q map: q = 32*(p%32) + p//32 (and its inverse)."""
    p = np.arange(N_FEAT)
    q_of_p = 32 * (p % 32) + p // 32
    p_of_q = np.argsort(q_of_p)
    return q_of_p, p_of_q


def _prep_weights(twiddle, bias):
    """Returns (wa, wb, ident, biasq) in device layouts (see kernel doc)."""
    tw = twiddle.astype(np.float64)
    eye = np.eye(N_FEAT)
    A = _apply_stages(eye, tw, range(5)).T        # A[p_out, p_in]
    B = _apply_stages(eye, tw, range(5, 10)).T    # B[p_out, p_in]
    q_of_p, p_of_q = _q_perm()
    Bq = B[np.ix_(p_of_q, p_of_q)]                # Bq[q_out, q_in]

    # wa[jl, c*128 + m], m = 16cp + 4u + w  ->  p_out = 128c + 32w + 4cp + u
    wa = np.zeros((128, 8 * 128))
    cp_, u_, w_ = np.meshgrid(np.arange(8), np.arange(4), np.arange(4),
                              indexing="ij")
    m_ = (16 * cp_ + 4 * u_ + w_).ravel()
    for c in range(8):
        pout = 128 * c + 32 * w_.ravel() + 4 * cp_.ravel() + u_.ravel()
        wa[:, c * 128 + m_] = A[np.ix_(pout, np.arange(128 * c, 128 * c + 128))].T
    # wb[k, cp*128 + k'] = Bq[128cp + k', 128cp + k]
    wb = np.zeros((128, 8 * 128))
    for cp in range(8):
        blk = Bq[128 * cp:128 * (cp + 1), 128 * cp:128 * (cp + 1)]
        wb[:, cp * 128:(cp + 1) * 128] = blk.T
    # biasq[k', cp] = bias[p_of_q[128cp + k']]
    biasq = np.zeros((128, 8), dtype=np.float32)
    for cp in range(8):
        biasq[:, cp] = bias[p_of_q[128 * cp:128 * (cp + 1)]]
    ident = np.eye(128, dtype=BF16)
    return (np.ascontiguousarray(wa.astype(BF16)),
            np.ascontiguousarray(wb.astype(BF16)),
            ident, biasq)


def _pack_xt(shard_bf):
    """x shard [bpc, 1024] bf16 -> block-major [128, 8*bpc]:
    xt[jl, k*1024 + c*128 + s] = x[k*128 + s, 128c + jl]."""
    bpc = shard_bf.shape[0]
    nblk = bpc // 128
    a = shard_bf.reshape(nblk, 128, 8, 128)        # [k, s, c, jl]
    return np.ascontiguousarray(
        a.transpose(3, 0, 2, 1).reshape(128, 8 * bpc)
    )


def _unpack_out(raw):
    """device out [128, 8*bpc] bf16 chunk-major (q-major features) ->
    [bpc, 1024] f32.  raw[k', j*8*CH + cp*CH + b*128 + s] =
    outq[128cp + k', (4j + b)*128 + s]."""
    bpc = raw.shape[1] // 8
    nch = bpc // CH
    _, p_of_q = _q_perm()
    q_of_p, _ = _q_perm()
    a = raw.reshape(128, nch, 8, NB, 128)          # [k', j, cp, b, s]
    outq = np.ascontiguousarray(
        a.transpose(2, 0, 1, 3, 4).reshape(N_FEAT, bpc)
    )
    # out[n, p] = outq[q_of_p[p], n]
    return outq[q_of_p, :].T.astype(np.float32)


# ---------------------------------------------------------------------------
# device program
# ---------------------------------------------------------------------------

_CACHE = {}


def _build_program(bpc, copy_plan=None, t_lag=2, b_lag=6,
                   psa_bufs=2, pst_bufs=2, psb_bufs=2):
    assert bpc % CH == 0
    nch = bpc // CH
    nblk = nch * NB
    nc = bacc.Bacc("TRN2", debug=False)
    xt_d = nc.dram_tensor("xt", [128, 8 * bpc], mybir.dt.bfloat16, kind="ExternalInput").ap()
    wa_d = nc.dram_tensor("wa", [128, 8 * 128], mybir.dt.bfloat16, kind="ExternalInput").ap()
    wb_d = nc.dram_tensor("wb", [128, 8 * 128], mybir.dt.bfloat16, kind="ExternalInput").ap()
    id_d = nc.dram_tensor("ident", [128, 128], mybir.dt.bfloat16, kind="ExternalInput").ap()
    bias_d = nc.dram_tensor("biasq", [128, 8], mybir.dt.float32, kind="ExternalInput").ap()
    out_d = nc.dram_tensor("outq", [128, 8 * bpc], mybir.dt.bfloat16, kind="ExternalOutput").ap()

    if copy_plan is None:
        copy_plan = {
            "A": ["dve", "act", "dve", "act"],     # per block (full [1024])
            "T": ["dve", "dve", "dve", "dve"],     # per block (full [1024], 2x)
            "B": ["act", "act", "act", "dve", "act", "act", "act", "act"],  # per cp
        }

    with tile.TileContext(nc) as tc:
        with (
            tc.tile_pool(name="w", bufs=1) as w_pool,
            tc.tile_pool(name="xin", bufs=max(8, nblk)) as xin_pool,
            tc.tile_pool(name="sa", bufs=4) as sa_pool,
            tc.tile_pool(name="saq", bufs=3) as saq_pool,
            tc.tile_pool(name="ot", bufs=3) as ot_pool,
            tc.tile_pool(name="psa", bufs=psa_bufs, space="PSUM") as psa_pool,
            tc.tile_pool(name="pst", bufs=pst_bufs, space="PSUM") as pst_pool,
            tc.tile_pool(name="psb", bufs=psb_bufs, space="PSUM") as psb_pool,
        ):
            wa = w_pool.tile([128, 8 * 128], mybir.dt.bfloat16, name="wa")
            wb = w_pool.tile([128, 8 * 128], mybir.dt.bfloat16, name="wb")
            ident = w_pool.tile([128, 128], mybir.dt.bfloat16, name="ident")
            biasq = w_pool.tile([128, 8], mybir.dt.float32, name="biasq")

            def eng(name):
                return {"dve": nc.vector, "act": nc.scalar, "pool": nc.gpsimd}[name]

            def copy_on(name, dst, src):
                if name == "act":
                    return nc.scalar.activation(dst, src, mybir.ActivationFunctionType.Identity)
                return eng(name).tensor_copy(dst, src)

            xins = {}

            def load(k):
                if k >= nblk:
                    return
                xin = xin_pool.tile([128, 1024], mybir.dt.bfloat16, name="xin")
                nc.sync.dma_start(xin[:], xt_d[:, k * 1024:(k + 1) * 1024])
                xins[k] = xin

            # interleaved weight + block loads: A(0) needs wa + load0 only
            nc.sync.dma_start(wa[:], wa_d[:])
            load(0)
            nc.sync.dma_start(ident[:], id_d[:])
            load(1)
            load(2)
            nc.sync.dma_start(biasq[:], bias_d[:])
            nc.sync.dma_start(wb[:], wb_d[:])
            for k in range(3, nblk):
                load(k)

            saqs = {}
            ots = {}
            sa2s = {}

            def a_block(k):
                xin = xins.pop(k)
                sa2 = sa_pool.tile([128, 1024], mybir.dt.bfloat16, name="sa2")
                psa = psa_pool.tile([128, 1024], mybir.dt.float32, name="psa", tag="ps")
                for c in range(8):
                    nc.tensor.matmul(
                        psa[:, c * 128:(c + 1) * 128],
                        xin[:, c * 128:(c + 1) * 128],
                        wa[:, c * 128:(c + 1) * 128],
                        start=True, stop=True,
                    )
                # permuting copy: sa2[:, 128cp+32u+4c+w] = psa[:, 128c+16cp+4u+w]
                in_ap = psa[:].rearrange("p (c cp u w) -> p c cp u w", c=8, cp=8, u=4, w=4)
                out_ap = sa2[:].rearrange("p (cp u c w) -> p cp u c w", cp=8, u=4, c=8, w=4)
                out_ap = out_ap.transpose([0, 3, 1, 2, 4])
                copy_on(copy_plan["A"][k % len(copy_plan["A"])], out_ap, in_ap)
                sa2s[k] = sa2

            def t_block(k):
                j, b = divmod(k, NB)
                sa2 = sa2s.pop(k)
                saq = saqs[j]
                pst = pst_pool.tile([128, 1024], mybir.dt.bfloat16, name="pst")
                for cp in range(8):
                    nc.tensor.transpose(
                        pst[:, cp * 128:(cp + 1) * 128],
                        sa2[:, cp * 128:(cp + 1) * 128],
                        ident[:],
                    )
                # copyT: pst[:, cp*128 + s] -> saq[:, cp*CH + b*128 + s]
                o_ap = saq[:].rearrange("p (cp n) -> p cp n", cp=8, n=CH)
                o_ap = o_ap[:, :, b * 128:(b + 1) * 128]
                i_ap = pst[:].rearrange("p (cp s) -> p cp s", cp=8, s=128)
                copy_on(copy_plan["T"][k % len(copy_plan["T"])], o_ap, i_ap)

            def b_half(pb, h, tail=False):
                j, cph = divmod(pb, NB)
                saq = saqs[j]
                ot = ots[j]
                cp = 2 * cph + h
                psb = psb_pool.tile([128, 512], mybir.dt.float32, name="psb")
                nc.tensor.matmul(
                    psb[:],
                    wb[:, cp * 128:(cp + 1) * 128],
                    saq[:, cp * CH:cp * CH + 512],
                    start=True, stop=True,
                )
                if tail:
                    e = ["act", "dve"][(2 * pb + h) % 2]
                else:
                    e = copy_plan["B"][(2 * pb + h) % len(copy_plan["B"])]
                dst = ot[:, cp * CH:cp * CH + 512]
                if e == "act":
                    nc.scalar.activation(
                        dst, psb[:], mybir.ActivationFunctionType.Identity,
                        bias=biasq[:, cp:cp + 1],
                    )
                else:
                    eng(e).tensor_scalar_add(dst, psb[:], biasq[:, cp:cp + 1])

            def b_pair(pb, tail=False):
                j, cph = divmod(pb, NB)
                for h in range(2):
                    b_half(pb, h, tail=tail)
                if j == nch - 1:
                    # last chunk: store each pair's slice immediately
                    ot = ots[j]
                    nc.sync.dma_start(
                        out_d[:, j * 8 * CH + cph * 2 * CH: j * 8 * CH + (cph + 1) * 2 * CH],
                        ot[:, cph * 2 * CH:(cph + 1) * 2 * CH],
                    )
                    if cph == NB - 1:
                        ots.pop(j)
                        saqs.pop(j)
                elif cph == NB - 1:
                    ot = ots.pop(j)
                    for hh in range(2):
                        nc.sync.dma_start(
                            out_d[:, j * 8 * CH + hh * 4 * CH: j * 8 * CH + (hh + 1) * 4 * CH],
                            ot[:, hh * 4 * CH:(hh + 1) * 4 * CH],
                        )
                    saqs.pop(j)

            nslot = max(nblk + t_lag, nblk - 1 + b_lag + 1)
            for k in range(nslot):
                if k < nblk:
                    j, b = divmod(k, NB)
                    if b == 0:
                        saqs[j] = saq_pool.tile([128, 8 * CH], mybir.dt.bfloat16, name="saq")
                        ots[j] = ot_pool.tile([128, 8 * CH], mybir.dt.bfloat16, name="ot")
                    a_block(k)
                if t_lag <= k < nblk + t_lag:
                    t_block(k - t_lag)
                if b_lag <= k < nblk + b_lag:
                    b_pair(k - b_lag, tail=(k >= nblk))

    nc.compile()
    return nc


# ---------------------------------------------------------------------------
# entry points
# ---------------------------------------------------------------------------

def kernel(x, twiddle, bias):
    global LAST_EXEC_NS
    batch = x.shape[0]
    assert batch % N_CORES == 0
    bpc = batch // N_CORES

    wa, wb, ident, biasq = _prep_weights(
        np.asarray(twiddle, dtype=np.float32), np.asarray(bias, dtype=np.float32)
    )
    x_bf = np.asarray(x).astype(BF16)
    shards = [
        _pack_xt(x_bf[k * bpc:(k + 1) * bpc, :]) for k in range(N_CORES)
    ]

    key = ("v3", bpc)
    if key not in _CACHE:
        _CACHE[key] = _build_program(bpc)
    nc = _CACHE[key]

    in_maps = [
        {"xt": shards[k], "wa": wa, "wb": wb, "ident": ident, "biasq": biasq}
        for k in range(N_CORES)
    ]
    try:
        res = run_bass_kernel_spmd(nc, in_maps, core_ids=list(range(N_CORES)))
    except ModuleNotFoundError:
        # BASS_TRACE set but the axon NTFF hook module isn't installed in
        # this container; retry with tracing force-disabled.
        os.environ["BASS_NEVER_TRACE"] = "1"
        res = run_bass_kernel_spmd(nc, in_maps, core_ids=list(range(N_CORES)))
    if res.exec_time_ns is not None:
        LAST_EXEC_NS = res.exec_time_ns

    out = np.empty((batch, N_FEAT), dtype=np.float32)
    for k in range(N_CORES):
        out[k * bpc:(k + 1) * bpc, :] = _unpack_out(res.results[k]["outq"])
    return out


def sim_time_ns(bpc=4096):
    """Deterministic single-core span from the instruction cost model
    (TimelineSim). All 8 cores run this same program in parallel."""
    from concourse.timeline_sim import TimelineSim

    key = ("v3", bpc)
    if key not in _CACHE:
        _CACHE[key] = _build_program(bpc)
    return TimelineSim(_CACHE[key], trace=False).simulate()


# revision 5
# speedup vs baseline: 1.8089x; 1.0312x over previous
"""Butterfly (nn_Butterfly) kernel for 8 Trainium2 NeuronCores.

Math: the 10 butterfly stages factor as out = B . (A . x) + bias where
A (stages 0-4) is block-diagonal over contiguous 32-blocks of p and
B (stages 5-9) is block-diagonal over contiguous 32-blocks of q, with
q = 32*(p % 32) + p//32.

Device pipeline per 128-sample block (three TensorE passes, no
SBUF<->SBUF DMA):
  pass A (flipped matmuls):  psa[samp, pfeat]  = x_blk^T . A^T   (per tile c)
  copyA (permuting):         sa2[samp, qfeat]  = psa permuted    (bf16)
  pass T (transposes):       pst[qfeat, samp]  = sa2^T           (per tile cp)
  copyT:                     saq[qfeat, samp]  (chunk-grouped)
  pass B (matmuls):          psb[qout, samp]   = B_cp . saq_cp
  copyB (+bias):             ot -> DMA out (bf16)

Batch (32768) is sharded across 8 cores; weights are replicated.
The host packs x into the transposed block-major device layout and
unpacks/un-permutes the q-major bf16 output back to f32 [batch, 1024].
"""

import os
import numpy as np
import ml_dtypes

import concourse.bass as bass
import concourse.bacc as bacc
import concourse.mybir as mybir
import concourse.tile as tile
from concourse.bass_utils import run_bass_kernel_spmd

N_FEAT = 1024
M_STAGES = 10
N_CORES = 8
NB = 4          # blocks per chunk
CH = NB * 128   # samples per chunk

BF16 = ml_dtypes.bfloat16

LAST_EXEC_NS = None  # set when BASS_KERNEL_TRACE=1

LABELS = {}  # instruction name -> label (for sim stall attribution)


def _lab(inst, label):
    try:
        LABELS[inst.ins.name] = label
    except Exception:
        pass
    return inst


# ---------------------------------------------------------------------------
# host-side weight/layout prep
# ---------------------------------------------------------------------------

def _apply_stages(x, twiddle, blocks):
    """Apply butterfly stages `blocks` to x [b, 1024] (mirrors reference)."""
    n = N_FEAT
    for m in blocks:
        s = 1 << m
        t = twiddle[0, m].reshape(n // (2 * s), s, 2, 2)
        o = x.reshape(-1, n // (2 * s), 2, s)
        x = np.einsum("gsij,bgjs->bgis", t, o).reshape(-1, n)
    return x


def _q_perm():
    """p -> q map: q = 32*(p%32) + p//32 (and its inverse)."""
    p = np.arange(N_FEAT)
    q_of_p = 32 * (p % 32) + p // 32
    p_of_q = np.argsort(q_of_p)
    return q_of_p, p_of_q


def _prep_weights(twiddle, bias):
    """Returns (wa, wb, ident, biasq) in device layouts (see kernel doc)."""
    tw = twiddle.astype(np.float64)
    eye = np.eye(N_FEAT)
    A = _apply_stages(eye, tw, range(5)).T        # A[p_out, p_in]
    B = _apply_stages(eye, tw, range(5, 10)).T    # B[p_out, p_in]
    q_of_p, p_of_q = _q_perm()
    Bq = B[np.ix_(p_of_q, p_of_q)]                # Bq[q_out, q_in]

    # wa[jl, c*128 + m], m = 16cp + 4u + w  ->  p_out = 128c + 32w + 4cp + u
    wa = np.zeros((128, 8 * 128))
    cp_, u_, w_ = np.meshgrid(np.arange(8), np.arange(4), np.arange(4),
                              indexing="ij")
    m_ = (16 * cp_ + 4 * u_ + w_).ravel()
    for c in range(8):
        pout = 128 * c + 32 * w_.ravel() + 4 * cp_.ravel() + u_.ravel()
        wa[:, c * 128 + m_] = A[np.ix_(pout, np.arange(128 * c, 128 * c + 128))].T
    # wb[k, cp*128 + k'] = Bq[128cp + k', 128cp + k]
    wb = np.zeros((128, 8 * 128))
    for cp in range(8):
        blk = Bq[128 * cp:128 * (cp + 1), 128 * cp:128 * (cp + 1)]
        wb[:, cp * 128:(cp + 1) * 128] = blk.T
    # biasq[k', cp] = bias[p_of_q[128cp + k']]
    biasq = np.zeros((128, 8), dtype=np.float32)
    for cp in range(8):
        biasq[:, cp] = bias[p_of_q[128 * cp:128 * (cp + 1)]]
    ident = np.eye(128, dtype=BF16)
    return (np.ascontiguousarray(wa.astype(BF16)),
            np.ascontiguousarray(wb.astype(BF16)),
            ident, biasq)


def _pack_xt(shard_bf):
    """x shard [bpc, 1024] bf16 -> block-major [128, 8*bpc]:
    xt[jl, k*1024 + c*128 + s] = x[k*128 + s, 128c + jl]."""
    bpc = shard_bf.shape[0]
    nblk = bpc // 128
    a = shard_bf.reshape(nblk, 128, 8, 128)        # [k, s, c, jl]
    return np.ascontiguousarray(
        a.transpose(3, 0, 2, 1).reshape(128, 8 * bpc)
    )


def _unpack_out(raw):
    """device out [128, 8*bpc] bf16 chunk-major (q-major features) ->
    [bpc, 1024] f32.  raw[k', j*8*CH + cp*CH + b*128 + s] =
    outq[128cp + k', (4j + b)*128 + s]."""
    bpc = raw.shape[1] // 8
    nch = bpc // CH
    _, p_of_q = _q_perm()
    q_of_p, _ = _q_perm()
    a = raw.reshape(128, nch, 8, NB, 128)          # [k', j, cp, b, s]
    outq = np.ascontiguousarray(
        a.transpose(2, 0, 1, 3, 4).reshape(N_FEAT, bpc)
    )
    # out[n, p] = outq[q_of_p[p], n]
    return outq[q_of_p, :].T.astype(np.float32)


# ---------------------------------------------------------------------------
# device program
# ---------------------------------------------------------------------------

_CACHE = {}


def _build_program(bpc, copy_plan=None, t_lag=2, b_lag=7,
                   psa_bufs=2, pst_bufs=2, psb_bufs=2, split_b=False):
    assert bpc % CH == 0
    nch = bpc // CH
    nblk = nch * NB
    nc = bacc.Bacc("TRN2", debug=False)
    xt_d = nc.dram_tensor("xt", [128, 8 * bpc], mybir.dt.bfloat16, kind="ExternalInput").ap()
    wa_d = nc.dram_tensor("wa", [128, 8 * 128], mybir.dt.bfloat16, kind="ExternalInput").ap()
    wb_d = nc.dram_tensor("wb", [128, 8 * 128], mybir.dt.bfloat16, kind="ExternalInput").ap()
    id_d = nc.dram_tensor("ident", [128, 128], mybir.dt.bfloat16, kind="ExternalInput").ap()
    bias_d = nc.dram_tensor("biasq", [128, 8], mybir.dt.float32, kind="ExternalInput").ap()
    out_d = nc.dram_tensor("outq", [128, 8 * bpc], mybir.dt.bfloat16, kind="ExternalOutput").ap()

    if copy_plan is None:
        copy_plan = {
            "A": ["dve", "act", "dve", "act"],     # per block (full [1024])
            "T": ["dve", "dve", "dve", "dve"],     # per block (full [1024], 2x)
            "B": ["act", "act", "act", "dve", "act", "act", "act", "act"],  # per cp
        }

    with tile.TileContext(nc) as tc:
        with (
            tc.tile_pool(name="w", bufs=1) as w_pool,
            tc.tile_pool(name="xin", bufs=max(8, nblk)) as xin_pool,
            tc.tile_pool(name="sa", bufs=4) as sa_pool,
            tc.tile_pool(name="saq", bufs=3) as saq_pool,
            tc.tile_pool(name="ot", bufs=3) as ot_pool,
            tc.tile_pool(name="psa", bufs=psa_bufs, space="PSUM") as psa_pool,
            tc.tile_pool(name="pst", bufs=pst_bufs, space="PSUM") as pst_pool,
            tc.tile_pool(name="psb", bufs=psb_bufs, space="PSUM") as psb_pool,
        ):
            wa = w_pool.tile([128, 8 * 128], mybir.dt.bfloat16, name="wa")
            wb = w_pool.tile([128, 8 * 128], mybir.dt.bfloat16, name="wb")
            ident = w_pool.tile([128, 128], mybir.dt.bfloat16, name="ident")
            biasq = w_pool.tile([128, 8], mybir.dt.float32, name="biasq")

            def eng(name):
                return {"dve": nc.vector, "act": nc.scalar, "pool": nc.gpsimd}[name]

            def copy_on(name, dst, src):
                if name == "act":
                    return nc.scalar.activation(dst, src, mybir.ActivationFunctionType.Identity)
                return eng(name).tensor_copy(dst, src)

            xins = {}

            def load(k):
                if k >= nblk:
                    return
                xin = xin_pool.tile([128, 1024], mybir.dt.bfloat16, name="xin")
                _lab(nc.sync.dma_start(xin[:], xt_d[:, k * 1024:(k + 1) * 1024]), f"load{k}")
                xins[k] = xin

            # interleaved weight + block loads: A(0) needs wa + load0 only
            nc.sync.dma_start(wa[:], wa_d[:])
            load(0)
            nc.sync.dma_start(ident[:], id_d[:])
            load(1)
            load(2)
            nc.sync.dma_start(biasq[:], bias_d[:])
            nc.sync.dma_start(wb[:], wb_d[:])
            for k in range(3, nblk):
                load(k)

            saqs = {}
            ots = {}
            sa2s = {}

            def a_block(k):
                xin = xins.pop(k)
                sa2 = sa_pool.tile([128, 1024], mybir.dt.bfloat16, name="sa2")
                psa = psa_pool.tile([128, 1024], mybir.dt.float32, name="psa", tag="ps")
                for c in range(8):
                    _lab(nc.tensor.matmul(
                        psa[:, c * 128:(c + 1) * 128],
                        xin[:, c * 128:(c + 1) * 128],
                        wa[:, c * 128:(c + 1) * 128],
                        start=True, stop=True,
                    ), f"A{k}c{c}")
                # permuting copy: sa2[:, 128cp+32u+4c+w] = psa[:, 128c+16cp+4u+w]
                in_ap = psa[:].rearrange("p (c cp u w) -> p c cp u w", c=8, cp=8, u=4, w=4)
                out_ap = sa2[:].rearrange("p (cp u c w) -> p cp u c w", cp=8, u=4, c=8, w=4)
                out_ap = out_ap.transpose([0, 3, 1, 2, 4])
                e = copy_plan["A"][k % len(copy_plan["A"])]
                _lab(copy_on(e, out_ap, in_ap), f"cA{k}:{e}")
                sa2s[k] = sa2

            def t_block(k):
                j, b = divmod(k, NB)
                sa2 = sa2s.pop(k)
                saq = saqs[j]
                pst = pst_pool.tile([128, 1024], mybir.dt.bfloat16, name="pst")
                for cp in range(8):
                    _lab(nc.tensor.transpose(
                        pst[:, cp * 128:(cp + 1) * 128],
                        sa2[:, cp * 128:(cp + 1) * 128],
                        ident[:],
                    ), f"T{k}c{cp}")
                # copyT: pst[:, cp*128 + s] -> saq[:, cp*CH + b*128 + s]
                o_ap = saq[:].rearrange("p (cp n) -> p cp n", cp=8, n=CH)
                o_ap = o_ap[:, :, b * 128:(b + 1) * 128]
                i_ap = pst[:].rearrange("p (cp s) -> p cp s", cp=8, s=128)
                e = copy_plan["T"][k % len(copy_plan["T"])]
                _lab(copy_on(e, o_ap, i_ap), f"cT{k}:{e}")

            def b_half(pb, h, tail=False):
                j, cph = divmod(pb, NB)
                saq = saqs[j]
                ot = ots[j]
                cp = 2 * cph + h
                pl = psa_pool if tail else psb_pool
                psb = pl.tile([128, 512], mybir.dt.float32, name="psb",
                              tag="ps" if tail else "")
                _lab(nc.tensor.matmul(
                    psb[:],
                    wb[:, cp * 128:(cp + 1) * 128],
                    saq[:, cp * CH:cp * CH + 512],
                    start=True, stop=True,
                ), f"B{pb}h{h}")
                if tail:
                    e = ["act", "dve"][(2 * pb + h) % 2]
                else:
                    e = copy_plan["B"][(2 * pb + h) % len(copy_plan["B"])]
                dst = ot[:, cp * CH:cp * CH + 512]
                if e == "act":
                    _lab(nc.scalar.activation(
                        dst, psb[:], mybir.ActivationFunctionType.Identity,
                        bias=biasq[:, cp:cp + 1],
                    ), f"cB{pb}h{h}:act")
                else:
                    _lab(eng(e).tensor_scalar_add(dst, psb[:], biasq[:, cp:cp + 1]), f"cB{pb}h{h}:{e}")

            def b_finish(pb):
                j, cph = divmod(pb, NB)
                if j == nch - 1:
                    # last chunk: store each pair's slice immediately
                    ot = ots[j]
                    nc.sync.dma_start(
                        out_d[:, j * 8 * CH + cph * 2 * CH: j * 8 * CH + (cph + 1) * 2 * CH],
                        ot[:, cph * 2 * CH:(cph + 1) * 2 * CH],
                    )
                    if cph == NB - 1:
                        ots.pop(j)
                        saqs.pop(j)
                elif cph == NB - 1:
                    ot = ots.pop(j)
                    for hh in range(2):
                        nc.sync.dma_start(
                            out_d[:, j * 8 * CH + hh * 4 * CH: j * 8 * CH + (hh + 1) * 4 * CH],
                            ot[:, hh * 4 * CH:(hh + 1) * 4 * CH],
                        )
                    saqs.pop(j)

            nslot = max(nblk + t_lag, nblk - 1 + b_lag + 1)
            for k in range(nslot):
                tl = k >= nblk
                in_b = b_lag <= k < nblk + b_lag
                if in_b and split_b:
                    b_half(k - b_lag, 0, tail=tl)
                if k < nblk:
                    j, b = divmod(k, NB)
                    if b == 0:
                        saqs[j] = saq_pool.tile([128, 8 * CH], mybir.dt.bfloat16, name="saq")
                        ots[j] = ot_pool.tile([128, 8 * CH], mybir.dt.bfloat16, name="ot")
                    a_block(k)
                if t_lag <= k < nblk + t_lag:
                    t_block(k - t_lag)
                if in_b:
                    if split_b:
                        b_half(k - b_lag, 1, tail=tl)
                    else:
                        b_half(k - b_lag, 0, tail=tl)
                        b_half(k - b_lag, 1, tail=tl)
                    b_finish(k - b_lag)

    nc.compile()
    return nc


# ---------------------------------------------------------------------------
# entry points
# ---------------------------------------------------------------------------

def kernel(x, twiddle, bias):
    global LAST_EXEC_NS
    batch = x.shape[0]
    assert batch % N_CORES == 0
    bpc = batch // N_CORES

    wa, wb, ident, biasq = _prep_weights(
        np.asarray(twiddle, dtype=np.float32), np.asarray(bias, dtype=np.float32)
    )
    x_bf = np.asarray(x).astype(BF16)
    shards = [
        _pack_xt(x_bf[k * bpc:(k + 1) * bpc, :]) for k in range(N_CORES)
    ]

    key = ("v3", bpc)
    if key not in _CACHE:
        _CACHE[key] = _build_program(bpc)
    nc = _CACHE[key]

    in_maps = [
        {"xt": shards[k], "wa": wa, "wb": wb, "ident": ident, "biasq": biasq}
        for k in range(N_CORES)
    ]
    try:
        res = run_bass_kernel_spmd(nc, in_maps, core_ids=list(range(N_CORES)))
    except ModuleNotFoundError:
        # BASS_TRACE set but the axon NTFF hook module isn't installed in
        # this container; retry with tracing force-disabled.
        os.environ["BASS_NEVER_TRACE"] = "1"
        res = run_bass_kernel_spmd(nc, in_maps, core_ids=list(range(N_CORES)))
    if res.exec_time_ns is not None:
        LAST_EXEC_NS = res.exec_time_ns

    out = np.empty((batch, N_FEAT), dtype=np.float32)
    for k in range(N_CORES):
        out[k * bpc:(k + 1) * bpc, :] = _unpack_out(res.results[k]["outq"])
    return out


def sim_time_ns(bpc=4096):
    """Deterministic single-core span from the instruction cost model
    (TimelineSim). All 8 cores run this same program in parallel."""
    from concourse.timeline_sim import TimelineSim

    key = ("v3", bpc)
    if key not in _CACHE:
        _CACHE[key] = _build_program(bpc)
    return TimelineSim(_CACHE[key], trace=False).simulate()


# revision 8
# speedup vs baseline: 1.8265x; 1.0097x over previous
"""Butterfly (nn_Butterfly) kernel for 8 Trainium2 NeuronCores.

Math: the 10 butterfly stages factor as out = B . (A . x) + bias where
A (stages 0-4) is block-diagonal over contiguous 32-blocks of p and
B (stages 5-9) is block-diagonal over contiguous 32-blocks of q, with
q = 32*(p % 32) + p//32.

Device pipeline per 128-sample block (three TensorE passes, no
SBUF<->SBUF DMA):
  pass A (flipped matmuls):  psa[samp, pfeat]  = x_blk^T . A^T   (per tile c)
  copyA (permuting):         sa2[samp, qfeat]  = psa permuted    (bf16)
  pass T (transposes):       pst[qfeat, samp]  = sa2^T           (per tile cp)
  copyT:                     saq[qfeat, samp]  (chunk-grouped)
  pass B (matmuls):          psb[qout, samp]   = B_cp . saq_cp
  copyB (+bias):             ot -> DMA out (bf16)

Batch (32768) is sharded across 8 cores; weights are replicated.
The host packs x into the transposed block-major device layout and
unpacks/un-permutes the q-major bf16 output back to f32 [batch, 1024].
"""

import os
import numpy as np
import ml_dtypes

import concourse.bass as bass
import concourse.bacc as bacc
import concourse.mybir as mybir
import concourse.tile as tile
from concourse.bass_utils import run_bass_kernel_spmd

N_FEAT = 1024
M_STAGES = 10
N_CORES = 8
NB = 4          # blocks per chunk
CH = NB * 128   # samples per chunk

BF16 = ml_dtypes.bfloat16

LAST_EXEC_NS = None  # set when BASS_KERNEL_TRACE=1

LABELS = {}  # instruction name -> label (for sim stall attribution)


def _lab(inst, label):
    try:
        LABELS[inst.ins.name] = label
    except Exception:
        pass
    return inst


# ---------------------------------------------------------------------------
# host-side weight/layout prep
# ---------------------------------------------------------------------------

def _apply_stages(x, twiddle, blocks):
    """Apply butterfly stages `blocks` to x [b, 1024] (mirrors reference)."""
    n = N_FEAT
    for m in blocks:
        s = 1 << m
        t = twiddle[0, m].reshape(n // (2 * s), s, 2, 2)
        o = x.reshape(-1, n // (2 * s), 2, s)
        x = np.einsum("gsij,bgjs->bgis", t, o).reshape(-1, n)
    return x


def _q_perm():
    """p -> q map: q = 32*(p%32) + p//32 (and its inverse)."""
    p = np.arange(N_FEAT)
    q_of_p = 32 * (p % 32) + p // 32
    p_of_q = np.argsort(q_of_p)
    return q_of_p, p_of_q


def _prep_weights(twiddle, bias):
    """Returns (wa, wb, ident, biasq) in device layouts (see kernel doc)."""
    tw = twiddle.astype(np.float64)
    eye = np.eye(N_FEAT)
    A = _apply_stages(eye, tw, range(5)).T        # A[p_out, p_in]
    B = _apply_stages(eye, tw, range(5, 10)).T    # B[p_out, p_in]
    q_of_p, p_of_q = _q_perm()
    Bq = B[np.ix_(p_of_q, p_of_q)]                # Bq[q_out, q_in]

    # wa[jl, c*128 + m], m = 16cp + 4u + w  ->  p_out = 128c + 32w + 4cp + u
    wa = np.zeros((128, 8 * 128))
    cp_, u_, w_ = np.meshgrid(np.arange(8), np.arange(4), np.arange(4),
                              indexing="ij")
    m_ = (16 * cp_ + 4 * u_ + w_).ravel()
    for c in range(8):
        pout = 128 * c + 32 * w_.ravel() + 4 * cp_.ravel() + u_.ravel()
        wa[:, c * 128 + m_] = A[np.ix_(pout, np.arange(128 * c, 128 * c + 128))].T
    # wb[k, cp*128 + k'] = Bq[128cp + k', 128cp + k]
    wb = np.zeros((128, 8 * 128))
    for cp in range(8):
        blk = Bq[128 * cp:128 * (cp + 1), 128 * cp:128 * (cp + 1)]
        wb[:, cp * 128:(cp + 1) * 128] = blk.T
    # biasq[k', cp] = bias[p_of_q[128cp + k']]
    biasq = np.zeros((128, 8), dtype=np.float32)
    for cp in range(8):
        biasq[:, cp] = bias[p_of_q[128 * cp:128 * (cp + 1)]]
    ident = np.eye(128, dtype=BF16)
    return (np.ascontiguousarray(wa.astype(BF16)),
            np.ascontiguousarray(wb.astype(BF16)),
            ident, biasq)


def _pack_xt(shard_bf):
    """x shard [bpc, 1024] bf16 -> block-major [128, 8*bpc]:
    xt[jl, k*1024 + c*128 + s] = x[k*128 + s, 128c + jl]."""
    bpc = shard_bf.shape[0]
    nblk = bpc // 128
    a = shard_bf.reshape(nblk, 128, 8, 128)        # [k, s, c, jl]
    return np.ascontiguousarray(
        a.transpose(3, 0, 2, 1).reshape(128, 8 * bpc)
    )


def _unpack_out(raw):
    """device out [128, 8*bpc] bf16 chunk-major (q-major features) ->
    [bpc, 1024] f32.  raw[k', j*8*CH + cp*CH + b*128 + s] =
    outq[128cp + k', (4j + b)*128 + s]."""
    bpc = raw.shape[1] // 8
    nch = bpc // CH
    _, p_of_q = _q_perm()
    q_of_p, _ = _q_perm()
    a = raw.reshape(128, nch, 8, NB, 128)          # [k', j, cp, b, s]
    outq = np.ascontiguousarray(
        a.transpose(2, 0, 1, 3, 4).reshape(N_FEAT, bpc)
    )
    # out[n, p] = outq[q_of_p[p], n]
    return outq[q_of_p, :].T.astype(np.float32)


# ---------------------------------------------------------------------------
# device program
# ---------------------------------------------------------------------------

_CACHE = {}


def _build_program(bpc, copy_plan=None, t_lag=2, b_lag=7,
                   psa_bufs=2, pst_bufs=2, psb_bufs=2, split_b=False):
    assert bpc % CH == 0
    nch = bpc // CH
    nblk = nch * NB
    nc = bacc.Bacc("TRN2", debug=False)
    xt_d = nc.dram_tensor("xt", [128, 8 * bpc], mybir.dt.bfloat16, kind="ExternalInput").ap()
    wa_d = nc.dram_tensor("wa", [128, 8 * 128], mybir.dt.bfloat16, kind="ExternalInput").ap()
    wb_d = nc.dram_tensor("wb", [128, 8 * 128], mybir.dt.bfloat16, kind="ExternalInput").ap()
    id_d = nc.dram_tensor("ident", [128, 128], mybir.dt.bfloat16, kind="ExternalInput").ap()
    bias_d = nc.dram_tensor("biasq", [128, 8], mybir.dt.float32, kind="ExternalInput").ap()
    out_d = nc.dram_tensor("outq", [128, 8 * bpc], mybir.dt.bfloat16, kind="ExternalOutput").ap()

    if copy_plan is None:
        copy_plan = {
            "A": ["dve", "act", "dve", "act"],     # per block (full [1024])
            "T": ["dve", "dve", "dve", "dve"],     # per block (full [1024], 2x)
            "B": ["act", "act", "act", "dve", "act", "act", "act", "act"],  # per cp
        }

    with tile.TileContext(nc) as tc:
        with (
            tc.tile_pool(name="w", bufs=1) as w_pool,
            tc.tile_pool(name="xin", bufs=max(8, nblk)) as xin_pool,
            tc.tile_pool(name="sa", bufs=6) as sa_pool,
            tc.tile_pool(name="saq", bufs=4) as saq_pool,
            tc.tile_pool(name="ot", bufs=4) as ot_pool,
            tc.tile_pool(name="psa", bufs=psa_bufs, space="PSUM") as psa_pool,
            tc.tile_pool(name="pst", bufs=pst_bufs, space="PSUM") as pst_pool,
            tc.tile_pool(name="psb", bufs=psb_bufs, space="PSUM") as psb_pool,
        ):
            wa = w_pool.tile([128, 8 * 128], mybir.dt.bfloat16, name="wa")
            wb = w_pool.tile([128, 8 * 128], mybir.dt.bfloat16, name="wb")
            ident = w_pool.tile([128, 128], mybir.dt.bfloat16, name="ident")
            biasq = w_pool.tile([128, 8], mybir.dt.float32, name="biasq")

            def eng(name):
                return {"dve": nc.vector, "act": nc.scalar, "pool": nc.gpsimd}[name]

            def copy_on(name, dst, src):
                if name == "act":
                    return nc.scalar.activation(dst, src, mybir.ActivationFunctionType.Identity)
                return eng(name).tensor_copy(dst, src)

            xins = {}

            def load(k):
                if k >= nblk:
                    return
                xin = xin_pool.tile([128, 1024], mybir.dt.bfloat16, name="xin")
                _lab(nc.sync.dma_start(xin[:], xt_d[:, k * 1024:(k + 1) * 1024]), f"load{k}")
                xins[k] = xin

            # lead-in: half-granular first loads so A(0) starts ASAP
            xin0 = xin_pool.tile([128, 1024], mybir.dt.bfloat16, name="xin")
            _lab(nc.sync.dma_start(xin0[:, :512], xt_d[:, :512]), "load0a")
            _lab(nc.sync.dma_start(wa[:, :512], wa_d[:, :512]), "wa_a")
            _lab(nc.sync.dma_start(xin0[:, 512:], xt_d[:, 512:1024]), "load0b")
            _lab(nc.sync.dma_start(wa[:, 512:], wa_d[:, 512:]), "wa_b")
            xins[0] = xin0
            nc.sync.dma_start(ident[:], id_d[:])
            load(1)
            load(2)
            nc.sync.dma_start(biasq[:], bias_d[:])
            nc.sync.dma_start(wb[:], wb_d[:])
            for k in range(3, nblk):
                load(k)

            saqs = {}
            ots = {}
            sa2s = {}

            def a_block(k):
                xin = xins.pop(k)
                sa2 = sa_pool.tile([128, 1024], mybir.dt.bfloat16, name="sa2")
                psa = psa_pool.tile([128, 1024], mybir.dt.float32, name="psa", tag="ps")
                for c in range(8):
                    _lab(nc.tensor.matmul(
                        psa[:, c * 128:(c + 1) * 128],
                        xin[:, c * 128:(c + 1) * 128],
                        wa[:, c * 128:(c + 1) * 128],
                        start=True, stop=True,
                    ), f"A{k}c{c}")
                # permuting copy: sa2[:, 128cp+32u+4c+w] = psa[:, 128c+16cp+4u+w]
                in_ap = psa[:].rearrange("p (c cp u w) -> p c cp u w", c=8, cp=8, u=4, w=4)
                out_ap = sa2[:].rearrange("p (cp u c w) -> p cp u c w", cp=8, u=4, c=8, w=4)
                out_ap = out_ap.transpose([0, 3, 1, 2, 4])
                e = copy_plan["A"][k % len(copy_plan["A"])]
                _lab(copy_on(e, out_ap, in_ap), f"cA{k}:{e}")
                sa2s[k] = sa2

            def t_block(k):
                j, b = divmod(k, NB)
                sa2 = sa2s.pop(k)
                saq = saqs[j]
                pst = pst_pool.tile([128, 1024], mybir.dt.bfloat16, name="pst")
                for cp in range(8):
                    _lab(nc.tensor.transpose(
                        pst[:, cp * 128:(cp + 1) * 128],
                        sa2[:, cp * 128:(cp + 1) * 128],
                        ident[:],
                    ), f"T{k}c{cp}")
                # copyT: pst[:, cp*128 + s] -> saq[:, cp*CH + b*128 + s]
                o_ap = saq[:].rearrange("p (cp n) -> p cp n", cp=8, n=CH)
                o_ap = o_ap[:, :, b * 128:(b + 1) * 128]
                i_ap = pst[:].rearrange("p (cp s) -> p cp s", cp=8, s=128)
                e = copy_plan["T"][k % len(copy_plan["T"])]
                _lab(copy_on(e, o_ap, i_ap), f"cT{k}:{e}")

            def b_half(pb, h, tail=False):
                j, cph = divmod(pb, NB)
                saq = saqs[j]
                ot = ots[j]
                cp = 2 * cph + h
                pl = psa_pool if tail else psb_pool
                psb = pl.tile([128, 512], mybir.dt.float32, name="psb",
                              tag="ps" if tail else "")
                _lab(nc.tensor.matmul(
                    psb[:],
                    wb[:, cp * 128:(cp + 1) * 128],
                    saq[:, cp * CH:cp * CH + 512],
                    start=True, stop=True,
                ), f"B{pb}h{h}")
                if tail:
                    e = ["act", "dve"][(2 * pb + h) % 2]
                else:
                    e = copy_plan["B"][(2 * pb + h) % len(copy_plan["B"])]
                dst = ot[:, cp * CH:cp * CH + 512]
                if e == "act":
                    _lab(nc.scalar.activation(
                        dst, psb[:], mybir.ActivationFunctionType.Identity,
                        bias=biasq[:, cp:cp + 1],
                    ), f"cB{pb}h{h}:act")
                else:
                    _lab(eng(e).tensor_scalar_add(dst, psb[:], biasq[:, cp:cp + 1]), f"cB{pb}h{h}:{e}")

            def b_finish(pb):
                j, cph = divmod(pb, NB)
                if j == nch - 1:
                    # last chunk: store each pair's slice immediately
                    ot = ots[j]
                    nc.sync.dma_start(
                        out_d[:, j * 8 * CH + cph * 2 * CH: j * 8 * CH + (cph + 1) * 2 * CH],
                        ot[:, cph * 2 * CH:(cph + 1) * 2 * CH],
                    )
                    if cph == NB - 1:
                        ots.pop(j)
                        saqs.pop(j)
                elif cph == NB - 1:
                    ot = ots.pop(j)
                    for hh in range(2):
                        nc.sync.dma_start(
                            out_d[:, j * 8 * CH + hh * 4 * CH: j * 8 * CH + (hh + 1) * 4 * CH],
                            ot[:, hh * 4 * CH:(hh + 1) * 4 * CH],
                        )
                    saqs.pop(j)

            import collections
            bq = collections.deque(range(nblk))  # pending B pairs
            nslot = nblk + t_lag

            def emit_pairs(n, tl):
                for _ in range(n):
                    if not bq:
                        return
                    pb = bq.popleft()
                    b_half(pb, 0, tail=tl)
                    b_half(pb, 1, tail=tl)
                    b_finish(pb)

            for k in range(nslot):
                if k < nblk:
                    j, b = divmod(k, NB)
                    if b == 0:
                        saqs[j] = saq_pool.tile([128, 8 * CH], mybir.dt.bfloat16, name="saq")
                        ots[j] = ot_pool.tile([128, 8 * CH], mybir.dt.bfloat16, name="ot")
                    a_block(k)
                if t_lag <= k < nblk + t_lag:
                    t_block(k - t_lag)
                if k >= b_lag:
                    emit_pairs(1, k >= nblk)
            # drain remaining pairs with the deepened psb pool
            while bq:
                emit_pairs(1, True)

    nc.compile()
    return nc


# ---------------------------------------------------------------------------
# entry points
# ---------------------------------------------------------------------------

def kernel(x, twiddle, bias):
    global LAST_EXEC_NS
    batch = x.shape[0]
    assert batch % N_CORES == 0
    bpc = batch // N_CORES

    wa, wb, ident, biasq = _prep_weights(
        np.asarray(twiddle, dtype=np.float32), np.asarray(bias, dtype=np.float32)
    )
    x_bf = np.asarray(x).astype(BF16)
    shards = [
        _pack_xt(x_bf[k * bpc:(k + 1) * bpc, :]) for k in range(N_CORES)
    ]

    key = ("v3", bpc)
    if key not in _CACHE:
        _CACHE[key] = _build_program(bpc)
    nc = _CACHE[key]

    in_maps = [
        {"xt": shards[k], "wa": wa, "wb": wb, "ident": ident, "biasq": biasq}
        for k in range(N_CORES)
    ]
    try:
        res = run_bass_kernel_spmd(nc, in_maps, core_ids=list(range(N_CORES)))
    except ModuleNotFoundError:
        # BASS_TRACE set but the axon NTFF hook module isn't installed in
        # this container; retry with tracing force-disabled.
        os.environ["BASS_NEVER_TRACE"] = "1"
        res = run_bass_kernel_spmd(nc, in_maps, core_ids=list(range(N_CORES)))
    if res.exec_time_ns is not None:
        LAST_EXEC_NS = res.exec_time_ns

    out = np.empty((batch, N_FEAT), dtype=np.float32)
    for k in range(N_CORES):
        out[k * bpc:(k + 1) * bpc, :] = _unpack_out(res.results[k]["outq"])
    return out


def sim_time_ns(bpc=4096):
    """Deterministic single-core span from the instruction cost model
    (TimelineSim). All 8 cores run this same program in parallel."""
    from concourse.timeline_sim import TimelineSim

    key = ("v3", bpc)
    if key not in _CACHE:
        _CACHE[key] = _build_program(bpc)
    return TimelineSim(_CACHE[key], trace=False).simulate()


# revision 9
# speedup vs baseline: 1.8397x; 1.0072x over previous
"""Butterfly (nn_Butterfly) kernel for 8 Trainium2 NeuronCores.

Math: the 10 butterfly stages factor as out = B . (A . x) + bias where
A (stages 0-4) is block-diagonal over contiguous 32-blocks of p and
B (stages 5-9) is block-diagonal over contiguous 32-blocks of q, with
q = 32*(p % 32) + p//32.

Device pipeline per 128-sample block (three TensorE passes, no
SBUF<->SBUF DMA):
  pass A (flipped matmuls):  psa[samp, pfeat]  = x_blk^T . A^T   (per tile c)
  copyA (permuting):         sa2[samp, qfeat]  = psa permuted    (bf16)
  pass T (transposes):       pst[qfeat, samp]  = sa2^T           (per tile cp)
  copyT:                     saq[qfeat, samp]  (chunk-grouped)
  pass B (matmuls):          psb[qout, samp]   = B_cp . saq_cp
  copyB (+bias):             ot -> DMA out (bf16)

Batch (32768) is sharded across 8 cores; weights are replicated.
The host packs x into the transposed block-major device layout and
unpacks/un-permutes the q-major bf16 output back to f32 [batch, 1024].
"""

import os
import numpy as np
import ml_dtypes

import concourse.bass as bass
import concourse.bacc as bacc
import concourse.mybir as mybir
import concourse.tile as tile
from concourse.bass_utils import run_bass_kernel_spmd

N_FEAT = 1024
M_STAGES = 10
N_CORES = 8
NB = 4          # blocks per chunk
CH = NB * 128   # samples per chunk

BF16 = ml_dtypes.bfloat16

LAST_EXEC_NS = None  # set when BASS_KERNEL_TRACE=1

LABELS = {}  # instruction name -> label (for sim stall attribution)


def _lab(inst, label):
    try:
        LABELS[inst.ins.name] = label
    except Exception:
        pass
    return inst


# ---------------------------------------------------------------------------
# host-side weight/layout prep
# ---------------------------------------------------------------------------

def _apply_stages(x, twiddle, blocks):
    """Apply butterfly stages `blocks` to x [b, 1024] (mirrors reference)."""
    n = N_FEAT
    for m in blocks:
        s = 1 << m
        t = twiddle[0, m].reshape(n // (2 * s), s, 2, 2)
        o = x.reshape(-1, n // (2 * s), 2, s)
        x = np.einsum("gsij,bgjs->bgis", t, o).reshape(-1, n)
    return x


def _q_perm():
    """p -> q map: q = 32*(p%32) + p//32 (and its inverse)."""
    p = np.arange(N_FEAT)
    q_of_p = 32 * (p % 32) + p // 32
    p_of_q = np.argsort(q_of_p)
    return q_of_p, p_of_q


def _prep_weights(twiddle, bias):
    """Returns (wa, wb, ident, biasq) in device layouts (see kernel doc)."""
    tw = twiddle.astype(np.float64)
    eye = np.eye(N_FEAT)
    A = _apply_stages(eye, tw, range(5)).T        # A[p_out, p_in]
    B = _apply_stages(eye, tw, range(5, 10)).T    # B[p_out, p_in]
    q_of_p, p_of_q = _q_perm()
    Bq = B[np.ix_(p_of_q, p_of_q)]                # Bq[q_out, q_in]

    # wa[jl, c*128 + m], m = 16cp + 4u + w  ->  p_out = 128c + 32w + 4cp + u
    wa = np.zeros((128, 8 * 128))
    cp_, u_, w_ = np.meshgrid(np.arange(8), np.arange(4), np.arange(4),
                              indexing="ij")
    m_ = (16 * cp_ + 4 * u_ + w_).ravel()
    for c in range(8):
        pout = 128 * c + 32 * w_.ravel() + 4 * cp_.ravel() + u_.ravel()
        wa[:, c * 128 + m_] = A[np.ix_(pout, np.arange(128 * c, 128 * c + 128))].T
    # wb[k, cp*128 + k'] = Bq[128cp + k', 128cp + k]
    wb = np.zeros((128, 8 * 128))
    for cp in range(8):
        blk = Bq[128 * cp:128 * (cp + 1), 128 * cp:128 * (cp + 1)]
        wb[:, cp * 128:(cp + 1) * 128] = blk.T
    # biasq[k', cp] = bias[p_of_q[128cp + k']]
    biasq = np.zeros((128, 8), dtype=np.float32)
    for cp in range(8):
        biasq[:, cp] = bias[p_of_q[128 * cp:128 * (cp + 1)]]
    ident = np.eye(128, dtype=BF16)
    return (np.ascontiguousarray(wa.astype(BF16)),
            np.ascontiguousarray(wb.astype(BF16)),
            ident, biasq)


def _pack_xt(shard_bf):
    """x shard [bpc, 1024] bf16 -> block-major [128, 8*bpc]:
    xt[jl, k*1024 + c*128 + s] = x[k*128 + s, 128c + jl]."""
    bpc = shard_bf.shape[0]
    nblk = bpc // 128
    a = shard_bf.reshape(nblk, 128, 8, 128)        # [k, s, c, jl]
    return np.ascontiguousarray(
        a.transpose(3, 0, 2, 1).reshape(128, 8 * bpc)
    )


def _unpack_out(raw):
    """device out [128, 8*bpc] bf16 chunk-major (q-major features) ->
    [bpc, 1024] f32.  raw[k', j*8*CH + cp*CH + b*128 + s] =
    outq[128cp + k', (4j + b)*128 + s]."""
    bpc = raw.shape[1] // 8
    nch = bpc // CH
    _, p_of_q = _q_perm()
    q_of_p, _ = _q_perm()
    a = raw.reshape(128, nch, 8, NB, 128)          # [k', j, cp, b, s]
    outq = np.ascontiguousarray(
        a.transpose(2, 0, 1, 3, 4).reshape(N_FEAT, bpc)
    )
    # out[n, p] = outq[q_of_p[p], n]
    return outq[q_of_p, :].T.astype(np.float32)


# ---------------------------------------------------------------------------
# device program
# ---------------------------------------------------------------------------

_CACHE = {}


def _build_program(bpc, copy_plan=None, t_lag=3, b_lag=7,
                   psa_bufs=2, pst_bufs=2, psb_bufs=2, split_b=False):
    assert bpc % CH == 0
    nch = bpc // CH
    nblk = nch * NB
    nc = bacc.Bacc("TRN2", debug=False)
    xt_d = nc.dram_tensor("xt", [128, 8 * bpc], mybir.dt.bfloat16, kind="ExternalInput").ap()
    wa_d = nc.dram_tensor("wa", [128, 8 * 128], mybir.dt.bfloat16, kind="ExternalInput").ap()
    wb_d = nc.dram_tensor("wb", [128, 8 * 128], mybir.dt.bfloat16, kind="ExternalInput").ap()
    id_d = nc.dram_tensor("ident", [128, 128], mybir.dt.bfloat16, kind="ExternalInput").ap()
    bias_d = nc.dram_tensor("biasq", [128, 8], mybir.dt.float32, kind="ExternalInput").ap()
    out_d = nc.dram_tensor("outq", [128, 8 * bpc], mybir.dt.bfloat16, kind="ExternalOutput").ap()

    if copy_plan is None:
        copy_plan = {
            "A": ["dve", "act", "dve", "act"],     # per block (full [1024])
            "T": ["dve", "dve", "dve", "dve"],     # per block (full [1024], 2x)
            "B": ["act", "act", "dve", "act", "act", "act", "dve", "act"],  # per cp
        }

    with tile.TileContext(nc) as tc:
        with (
            tc.tile_pool(name="w", bufs=1) as w_pool,
            tc.tile_pool(name="xin", bufs=max(8, nblk)) as xin_pool,
            tc.tile_pool(name="sa", bufs=6) as sa_pool,
            tc.tile_pool(name="saq", bufs=4) as saq_pool,
            tc.tile_pool(name="ot", bufs=4) as ot_pool,
            tc.tile_pool(name="psa", bufs=psa_bufs, space="PSUM") as psa_pool,
            tc.tile_pool(name="pst", bufs=pst_bufs, space="PSUM") as pst_pool,
            tc.tile_pool(name="psb", bufs=psb_bufs, space="PSUM") as psb_pool,
        ):
            wa = w_pool.tile([128, 8 * 128], mybir.dt.bfloat16, name="wa")
            wb = w_pool.tile([128, 8 * 128], mybir.dt.bfloat16, name="wb")
            ident = w_pool.tile([128, 128], mybir.dt.bfloat16, name="ident")
            biasq = w_pool.tile([128, 8], mybir.dt.float32, name="biasq")

            def eng(name):
                return {"dve": nc.vector, "act": nc.scalar, "pool": nc.gpsimd}[name]

            def copy_on(name, dst, src):
                if name == "act":
                    return nc.scalar.activation(dst, src, mybir.ActivationFunctionType.Identity)
                return eng(name).tensor_copy(dst, src)

            xins = {}

            def load(k):
                if k >= nblk:
                    return
                xin = xin_pool.tile([128, 1024], mybir.dt.bfloat16, name="xin")
                _lab(nc.sync.dma_start(xin[:], xt_d[:, k * 1024:(k + 1) * 1024]), f"load{k}")
                xins[k] = xin

            # lead-in: half-granular first loads so A(0) starts ASAP
            xin0 = xin_pool.tile([128, 1024], mybir.dt.bfloat16, name="xin")
            _lab(nc.sync.dma_start(xin0[:, :512], xt_d[:, :512]), "load0a")
            _lab(nc.sync.dma_start(wa[:, :512], wa_d[:, :512]), "wa_a")
            _lab(nc.sync.dma_start(xin0[:, 512:], xt_d[:, 512:1024]), "load0b")
            _lab(nc.sync.dma_start(wa[:, 512:], wa_d[:, 512:]), "wa_b")
            xins[0] = xin0
            nc.sync.dma_start(ident[:], id_d[:])
            load(1)
            load(2)
            nc.sync.dma_start(biasq[:], bias_d[:])
            nc.sync.dma_start(wb[:], wb_d[:])
            for k in range(3, nblk):
                load(k)

            saqs = {}
            ots = {}
            sa2s = {}

            def a_block(k):
                xin = xins.pop(k)
                sa2 = sa_pool.tile([128, 1024], mybir.dt.bfloat16, name="sa2")
                psa = psa_pool.tile([128, 1024], mybir.dt.float32, name="psa", tag="ps")
                for c in range(8):
                    _lab(nc.tensor.matmul(
                        psa[:, c * 128:(c + 1) * 128],
                        xin[:, c * 128:(c + 1) * 128],
                        wa[:, c * 128:(c + 1) * 128],
                        start=True, stop=True,
                    ), f"A{k}c{c}")
                # permuting copy: sa2[:, 128cp+32u+4c+w] = psa[:, 128c+16cp+4u+w]
                in_ap = psa[:].rearrange("p (c cp u w) -> p c cp u w", c=8, cp=8, u=4, w=4)
                out_ap = sa2[:].rearrange("p (cp u c w) -> p cp u c w", cp=8, u=4, c=8, w=4)
                out_ap = out_ap.transpose([0, 3, 1, 2, 4])
                e = copy_plan["A"][k % len(copy_plan["A"])]
                _lab(copy_on(e, out_ap, in_ap), f"cA{k}:{e}")
                sa2s[k] = sa2

            def t_block(k):
                j, b = divmod(k, NB)
                sa2 = sa2s.pop(k)
                saq = saqs[j]
                pst = pst_pool.tile([128, 1024], mybir.dt.bfloat16, name="pst")
                for cp in range(8):
                    _lab(nc.tensor.transpose(
                        pst[:, cp * 128:(cp + 1) * 128],
                        sa2[:, cp * 128:(cp + 1) * 128],
                        ident[:],
                    ), f"T{k}c{cp}")
                # copyT: pst[:, cp*128 + s] -> saq[:, cp*CH + b*128 + s]
                o_ap = saq[:].rearrange("p (cp n) -> p cp n", cp=8, n=CH)
                o_ap = o_ap[:, :, b * 128:(b + 1) * 128]
                i_ap = pst[:].rearrange("p (cp s) -> p cp s", cp=8, s=128)
                e = copy_plan["T"][k % len(copy_plan["T"])]
                _lab(copy_on(e, o_ap, i_ap), f"cT{k}:{e}")

            def b_half(pb, h, tail=False):
                j, cph = divmod(pb, NB)
                saq = saqs[j]
                ot = ots[j]
                cp = 2 * cph + h
                pl = psa_pool if tail else psb_pool
                psb = pl.tile([128, 512], mybir.dt.float32, name="psb",
                              tag="ps" if tail else "")
                _lab(nc.tensor.matmul(
                    psb[:],
                    wb[:, cp * 128:(cp + 1) * 128],
                    saq[:, cp * CH:cp * CH + 512],
                    start=True, stop=True,
                ), f"B{pb}h{h}")
                if tail:
                    e = ["act", "dve"][(2 * pb + h) % 2]
                else:
                    e = copy_plan["B"][(2 * pb + h) % len(copy_plan["B"])]
                dst = ot[:, cp * CH:cp * CH + 512]
                if e == "act":
                    _lab(nc.scalar.activation(
                        dst, psb[:], mybir.ActivationFunctionType.Identity,
                        bias=biasq[:, cp:cp + 1],
                    ), f"cB{pb}h{h}:act")
                else:
                    _lab(eng(e).tensor_scalar_add(dst, psb[:], biasq[:, cp:cp + 1]), f"cB{pb}h{h}:{e}")

            def b_finish(pb):
                j, cph = divmod(pb, NB)
                if j == nch - 1:
                    # last chunk: store each pair's slice immediately
                    ot = ots[j]
                    nc.sync.dma_start(
                        out_d[:, j * 8 * CH + cph * 2 * CH: j * 8 * CH + (cph + 1) * 2 * CH],
                        ot[:, cph * 2 * CH:(cph + 1) * 2 * CH],
                    )
                    if cph == NB - 1:
                        ots.pop(j)
                        saqs.pop(j)
                elif cph == NB - 1:
                    ot = ots.pop(j)
                    for hh in range(2):
                        nc.sync.dma_start(
                            out_d[:, j * 8 * CH + hh * 4 * CH: j * 8 * CH + (hh + 1) * 4 * CH],
                            ot[:, hh * 4 * CH:(hh + 1) * 4 * CH],
                        )
                    saqs.pop(j)

            import collections
            bq = collections.deque(range(nblk))  # pending B pairs
            nslot = nblk + t_lag

            def emit_pairs(n, tl):
                for _ in range(n):
                    if not bq:
                        return
                    pb = bq.popleft()
                    b_half(pb, 0, tail=tl)
                    b_half(pb, 1, tail=tl)
                    b_finish(pb)

            for k in range(nslot):
                if k < nblk:
                    j, b = divmod(k, NB)
                    if b == 0:
                        saqs[j] = saq_pool.tile([128, 8 * CH], mybir.dt.bfloat16, name="saq")
                        ots[j] = ot_pool.tile([128, 8 * CH], mybir.dt.bfloat16, name="ot")
                    a_block(k)
                if t_lag <= k < nblk + t_lag:
                    t_block(k - t_lag)
                if k >= b_lag:
                    emit_pairs(1, k >= nblk)
            # drain remaining pairs with the deepened psb pool
            while bq:
                emit_pairs(1, True)

    nc.compile()
    return nc


# ---------------------------------------------------------------------------
# entry points
# ---------------------------------------------------------------------------

def kernel(x, twiddle, bias):
    global LAST_EXEC_NS
    batch = x.shape[0]
    assert batch % N_CORES == 0
    bpc = batch // N_CORES

    wa, wb, ident, biasq = _prep_weights(
        np.asarray(twiddle, dtype=np.float32), np.asarray(bias, dtype=np.float32)
    )
    x_bf = np.asarray(x).astype(BF16)
    shards = [
        _pack_xt(x_bf[k * bpc:(k + 1) * bpc, :]) for k in range(N_CORES)
    ]

    key = ("v3", bpc)
    if key not in _CACHE:
        _CACHE[key] = _build_program(bpc)
    nc = _CACHE[key]

    in_maps = [
        {"xt": shards[k], "wa": wa, "wb": wb, "ident": ident, "biasq": biasq}
        for k in range(N_CORES)
    ]
    try:
        res = run_bass_kernel_spmd(nc, in_maps, core_ids=list(range(N_CORES)))
    except ModuleNotFoundError:
        # BASS_TRACE set but the axon NTFF hook module isn't installed in
        # this container; retry with tracing force-disabled.
        os.environ["BASS_NEVER_TRACE"] = "1"
        res = run_bass_kernel_spmd(nc, in_maps, core_ids=list(range(N_CORES)))
    if res.exec_time_ns is not None:
        LAST_EXEC_NS = res.exec_time_ns

    out = np.empty((batch, N_FEAT), dtype=np.float32)
    for k in range(N_CORES):
        out[k * bpc:(k + 1) * bpc, :] = _unpack_out(res.results[k]["outq"])
    return out


def sim_time_ns(bpc=4096):
    """Deterministic single-core span from the instruction cost model
    (TimelineSim). All 8 cores run this same program in parallel."""
    from concourse.timeline_sim import TimelineSim

    key = ("v3", bpc)
    if key not in _CACHE:
        _CACHE[key] = _build_program(bpc)
    return TimelineSim(_CACHE[key], trace=False).simulate()
